# revision 19
# baseline (speedup 1.0000x reference)
"""DeltaNet Trainium2 kernel — 8-core SPMD, one (batch, head) pair per core.

Full inputs -> shard on host -> Bass/Tile kernel per core -> host unshard.

Per-core pipeline (b = core//4, h = core%4):
  xt = X[b]^T resident in SBUF; q/k/v head projections fused with causal
  conv (diagonal-stationary matmuls) and SiLU; l2-norm and beta folded into
  per-token scalars (alpha_q deferred to output, alpha_k/beta folded into
  row scales of the chunked delta-rule); chunked delta rule (C=128) with the
  (I+A)^-1 triangular solve applied via a depth-4 Neumann product directly
  to the rhs; per-chunk RMS-norm and partial o_proj.  Host sums the 4
  per-head partial o_proj outputs per batch.
"""

import os
import sys
from contextlib import ExitStack

import numpy as np

for _p in ("/opt/trn_rl_repo", "/root/.axon_site/_ro/trn_rl_repo"):
    if os.path.isdir(_p) and _p not in sys.path:
        sys.path.insert(0, _p)

import concourse.bass as bass  # noqa: E402
import concourse.tile as tile  # noqa: E402
from concourse import bacc, mybir  # noqa: E402
from concourse.bass_utils import run_bass_kernel_spmd  # noqa: E402

F32 = mybir.dt.float32
F32R = mybir.dt.float32r
AF = mybir.ActivationFunctionType
OP = mybir.AluOpType

HID = 1024
D = 256
C = 128
KT = HID // 128  # 8 k-tiles over the hidden contraction dim
NH = 4
B = 2
S_FULL = 2048


def build_nc(nchunk=S_FULL // C, dbg=False):
    S = nchunk * C
    scs = 512 if S >= 512 else S
    nsc = S // scs
    nc = bacc.Bacc("TRN2", target_bir_lowering=False, debug=False)

    xt_d = nc.dram_tensor("xt", [HID, S], F32R, kind="ExternalInput")
    wq_d = nc.dram_tensor("wq", [HID, D], F32R, kind="ExternalInput")
    wk_d = nc.dram_tensor("wk", [HID, D], F32R, kind="ExternalInput")
    wv_d = nc.dram_tensor("wv", [HID, D], F32R, kind="ExternalInput")
    wb_d = nc.dram_tensor("wb", [HID, 1], F32R, kind="ExternalInput")
    wo_d = nc.dram_tensor("wo", [D, HID], F32R, kind="ExternalInput")
    cdq_d = nc.dram_tensor("cdq", [128, 8 * 128], F32R, kind="ExternalInput")
    cdk_d = nc.dram_tensor("cdk", [128, 8 * 128], F32R, kind="ExternalInput")
    cdv_d = nc.dram_tensor("cdv", [128, 8 * 128], F32R, kind="ExternalInput")
    ident_d = nc.dram_tensor("ident", [128, 128], F32R, kind="ExternalInput")
    onescol_d = nc.dram_tensor("onescol", [128, 1], F32R, kind="ExternalInput")
    mlow_d = nc.dram_tensor("mlow", [128, 128], F32, kind="ExternalInput")
    mup_d = nc.dram_tensor("mup", [128, 128], F32, kind="ExternalInput")
    out_d = nc.dram_tensor("out", [S, HID], F32, kind="ExternalOutput")
    dbg_d = {}
    if dbg:
        for t in ("q", "k", "v"):
            for dt_ in range(2):
                dbg_d[f"{t}{dt_}"] = nc.dram_tensor(
                    f"dbg_{t}{dt_}", [128, S], F32, kind="ExternalOutput"
                )
        for r in ("bk", "nbk2", "aq"):
            dbg_d[r] = nc.dram_tensor(f"dbg_{r}", [1, S], F32, kind="ExternalOutput")
        dbg_d["tok"] = nc.dram_tensor("dbg_tok", [128, 4 * nchunk], F32, kind="ExternalOutput")
        dbg_d["w"] = nc.dram_tensor("dbg_w", [128, 256 * nchunk], F32, kind="ExternalOutput")
        dbg_d["on"] = nc.dram_tensor("dbg_on", [128, 256 * nchunk], F32, kind="ExternalOutput")
        dbg_d["X"] = nc.dram_tensor("dbg_X", [128, 128 * nchunk], F32, kind="ExternalOutput")
        dbg_d["S"] = nc.dram_tensor("dbg_S", [128, 512 * nchunk], F32, kind="ExternalOutput")
    dbg_d = {}
    if dbg:
        for t in ("q", "k", "v"):
            for dt_ in range(2):
                dbg_d[f"{t}{dt_}"] = nc.dram_tensor(
                    f"dbg_{t}{dt_}", [128, S], F32, kind="ExternalOutput"
                )
        for r in ("bk", "nbk2", "aq"):
            dbg_d[r] = nc.dram_tensor(f"dbg_{r}", [1, S], F32, kind="ExternalOutput")
        dbg_d["tok"] = nc.dram_tensor("dbg_tok", [128, 4 * nchunk], F32, kind="ExternalOutput")
        dbg_d["w"] = nc.dram_tensor("dbg_w", [128, 256 * nchunk], F32, kind="ExternalOutput")
        dbg_d["on"] = nc.dram_tensor("dbg_on", [128, 256 * nchunk], F32, kind="ExternalOutput")
        dbg_d["X"] = nc.dram_tensor("dbg_X", [128, 128 * nchunk], F32, kind="ExternalOutput")
        dbg_d["S"] = nc.dram_tensor("dbg_S", [128, 512 * nchunk], F32, kind="ExternalOutput")

    with tile.TileContext(nc) as tc, ExitStack() as ctx:
        # ---------------- persistent pools ----------------
        pmask = ctx.enter_context(tc.tile_pool(name="pmask", bufs=1))
        prow = ctx.enter_context(tc.tile_pool(name="prow", bufs=1))
        pplane = ctx.enter_context(tc.tile_pool(name="pplane", bufs=1))
        pwo = ctx.enter_context(tc.tile_pool(name="pwo", bufs=1))

        ident = pmask.tile([128, 128], F32R)
        onescol = pmask.tile([128, 1], F32R)
        mlow = pmask.tile([128, 128], F32)
        mup = pmask.tile([128, 128], F32)
        nc.sync.dma_start(out=ident, in_=ident_d.ap())
        nc.sync.dma_start(out=onescol, in_=onescol_d.ap())
        nc.sync.dma_start(out=mlow, in_=mlow_d.ap())
        nc.sync.dma_start(out=mup, in_=mup_d.ap())
        eps6 = pmask.tile([128, 1], F32)
        nc.vector.memset(eps6, 1e-6)
        eps5 = pmask.tile([128, 1], F32)
        nc.vector.memset(eps5, 1e-5)

        # packed per-token scale rows {bk, nbk2, aq} for per-chunk transposes
        rows4 = prow.tile([4, S], F32R)

        wo_sb = pwo.tile([128, 2, HID], F32R)
        nc.sync.dma_start(
            out=wo_sb, in_=wo_d.ap().rearrange("(t p) h -> p t h", p=128)
        )

        # q/k/v planes, 2 d-tiles each (post conv+silu, raw scales)
        planes = {}
        for t in ("q", "k", "v"):
            for dt_ in range(2):
                planes[(t, dt_)] = pplane.tile(
                    [128, S], F32R, tag=f"plane_{t}{dt_}", name=f"plane_{t}{dt_}"
                )

        # ---------------- phase B: projections + conv + silu + beta --------
        with ExitStack() as bc2:
            pbc = bc2.enter_context(tc.tile_pool(name="pbc", bufs=1))
            beta_row = pbc.tile([1, S], F32R)
            ak_row = pbc.tile([1, S], F32R)
            bk_row = pbc.tile([1, S], F32R)    # beta * alpha_k
            nbk2_row = pbc.tile([1, S], F32R)  # -beta * alpha_k^2
            aq_row = pbc.tile([1, S], F32R)    # alpha_q

            with ExitStack() as bctx:
                pxt = bctx.enter_context(tc.tile_pool(name="pxt", bufs=1))
                pw = bctx.enter_context(tc.tile_pool(name="pw", bufs=1))
                pdiag = bctx.enter_context(tc.tile_pool(name="pdiag", bufs=1))
                praw = bctx.enter_context(tc.tile_pool(name="praw", bufs=1))
                ppt_b = bctx.enter_context(
                    tc.tile_pool(name="pptb", bufs=5, space="PSUM")
                )

                xt_sb = pxt.tile([128, KT, S], F32R)
                nc.sync.dma_start(
                    out=xt_sb, in_=xt_d.ap().rearrange("(k p) s -> p k s", p=128)
                )
                wb_sb = pw.tile([128, KT, 1], F32R, tag="wb")
                nc.sync.dma_start(
                    out=wb_sb, in_=wb_d.ap().rearrange("(k p) o -> p k o", p=128)
                )

                # beta row
                for sc in range(nsc):
                    psb = ppt_b.tile([1, scs], F32, tag="ps", name="psb")
                    for kk in range(KT):
                        nc.tensor.matmul(
                            psb,
                            wb_sb[:, kk, :],
                            xt_sb[:, kk, sc * scs : (sc + 1) * scs],
                            start=(kk == 0),
                            stop=(kk == KT - 1),
                        )
                    nc.scalar.activation(
                        out=beta_row[0:1, sc * scs : (sc + 1) * scs],
                        in_=psb,
                        func=AF.Sigmoid,
                    )

                wd = {"q": (wq_d, cdq_d), "k": (wk_d, cdk_d), "v": (wv_d, cdv_d)}
                copy_flip = 0
                for t in ("q", "k", "v"):
                    w_d, cd_d = wd[t]
                    w_sb = pw.tile([128, KT, D], F32R, tag="w", name=f"w_{t}")
                    nc.sync.dma_start(
                        out=w_sb, in_=w_d.ap().rearrange("(k p) d -> p k d", p=128)
                    )
                    diag = pdiag.tile(
                        [128, 8 * 128], F32R, tag="diag", name=f"diag_{t}"
                    )
                    nc.sync.dma_start(out=diag, in_=cd_d.ap())
                    for dt_ in range(2):
                        raw = praw.tile(
                            [128, S + 8], F32R, tag="raw", name=f"raw_{t}{dt_}"
                        )
                        nc.gpsimd.memset(raw[:, 0:8].bitcast(F32), 0.0)
                        for sc in range(nsc):
                            ps = ppt_b.tile([128, scs], F32, tag="ps", name="psraw")
                            for kk in range(KT):
                                nc.tensor.matmul(
                                    ps,
                                    w_sb[:, kk, dt_ * 128 : (dt_ + 1) * 128],
                                    xt_sb[:, kk, sc * scs : (sc + 1) * scs],
                                    start=(kk == 0),
                                    stop=(kk == KT - 1),
                                )
                            dst = raw[:, 8 + sc * scs : 8 + (sc + 1) * scs]
                            if copy_flip % 2 == 0:
                                nc.scalar.activation(out=dst, in_=ps, func=AF.Copy)
                            else:
                                nc.vector.tensor_copy(dst, ps)
                            copy_flip += 1
                        # conv (4 taps as diagonal-stationary matmuls) + SiLU
                        plane = planes[(t, dt_)]
                        for sc in range(nsc):
                            base = sc * scs
                            psc = ppt_b.tile([128, scs], F32, tag="ps", name="psconv")
                            for j in (3, 2, 1, 0):
                                sh = 3 - j
                                dslc = diag[
                                    :, (j * 2 + dt_) * 128 : (j * 2 + dt_ + 1) * 128
                                ]
                                nc.tensor.matmul(
                                    psc,
                                    dslc,
                                    raw[:, 8 + base - sh : 8 + base + scs - sh],
                                    start=(j == 3),
                                    stop=(j == 0),
                                )
                            nc.scalar.activation(
                                out=plane[:, base : base + scs], in_=psc, func=AF.Silu
                            )

            # ---------------- phase C: l2 rows ----------------
            with ExitStack() as cctx:
                psq = cctx.enter_context(tc.tile_pool(name="psq", bufs=3))
                pscr = cctx.enter_context(tc.tile_pool(name="pscr", bufs=2))
                ppt_c = cctx.enter_context(
                    tc.tile_pool(name="pptc", bufs=4, space="PSUM")
                )
                for t, dest in (("q", aq_row), ("k", ak_row)):
                    for sc in range(nsc):
                        sl = slice(sc * scs, (sc + 1) * scs)
                        psl = ppt_c.tile([1, scs], F32, tag="ps", name="psl")
                        for dt_ in range(2):
                            sq = psq.tile([128, scs], F32R, tag="sq", name="sq")
                            src = planes[(t, dt_)][:, sl]
                            if dt_ == 0:
                                nc.vector.tensor_mul(sq, src, src)
                            else:
                                nc.scalar.activation(out=sq, in_=src, func=AF.Square)
                            nc.tensor.matmul(
                                psl, onescol, sq, start=(dt_ == 0), stop=(dt_ == 1)
                            )
                        scr = pscr.tile([1, scs], F32, tag="scr", name="scr")
                        nc.scalar.activation(
                            out=scr, in_=psl, func=AF.Sqrt, bias=eps6[0:1, :]
                        )
                        with nc.allow_low_precision(reason="f32r row scales"):
                            nc.vector.reciprocal(out=dest[0:1, sl], in_=scr)
                # bk = beta*ak ; nbk2 = -bk*ak
                with nc.allow_low_precision(reason="f32r row scales"):
                    nc.vector.tensor_mul(bk_row, beta_row, ak_row)
                    nc.vector.scalar_tensor_tensor(
                        out=nbk2_row,
                        in0=bk_row,
                        scalar=-1.0,
                        in1=ak_row,
                        op0=OP.mult,
                        op1=OP.mult,
                    )

            if dbg:
                for t in ("q", "k", "v"):
                    for dt_ in range(2):
                        nc.sync.dma_start(
                            out=dbg_d[f"{t}{dt_}"].ap(),
                            in_=planes[(t, dt_)][:, :].bitcast(F32),
                        )
                nc.sync.dma_start(out=dbg_d["bk"].ap(), in_=bk_row[0:1, :].bitcast(F32))
                nc.sync.dma_start(
                    out=dbg_d["nbk2"].ap(), in_=nbk2_row[0:1, :].bitcast(F32)
                )
                nc.sync.dma_start(out=dbg_d["aq"].ap(), in_=aq_row[0:1, :].bitcast(F32))

            # pack the three scale rows into one tile (DMA may write any partition)
            nc.sync.dma_start(out=rows4[0:1, :], in_=bk_row[0:1, :])
            nc.sync.dma_start(out=rows4[1:2, :], in_=nbk2_row[0:1, :])
            nc.sync.dma_start(out=rows4[2:3, :], in_=aq_row[0:1, :])

        if dbg:
            for t in ("q", "k", "v"):
                for dt_ in range(2):
                    nc.sync.dma_start(out=dbg_d[f"{t}{dt_}"].ap(), in_=planes[(t, dt_)][:, :].bitcast(F32))
            nc.sync.dma_start(out=dbg_d["bk"].ap(), in_=bk_row[0:1, :].bitcast(F32))
            nc.sync.dma_start(out=dbg_d["nbk2"].ap(), in_=nbk2_row[0:1, :].bitcast(F32))
            nc.sync.dma_start(out=dbg_d["aq"].ap(), in_=aq_row[0:1, :].bitcast(F32))


        # ---------------- phase D: chunked delta rule ----------------
        pS = ctx.enter_context(tc.tile_pool(name="pS", bufs=2))
        pcs = ctx.enter_context(tc.tile_pool(name="pcs", bufs=2))
        pcm = ctx.enter_context(tc.tile_pool(name="pcm", bufs=2))
        ptok = ctx.enter_context(tc.tile_pool(name="ptok", bufs=2))
        pout = ctx.enter_context(tc.tile_pool(name="pout", bufs=2))
        ppS = ctx.enter_context(tc.tile_pool(name="ppS", bufs=1, space="PSUM"))
        ppop = ctx.enter_context(tc.tile_pool(name="ppop", bufs=2, space="PSUM"))
        ppt = ctx.enter_context(tc.tile_pool(name="ppt", bufs=4, space="PSUM"))

        psS0 = ppS.tile([128, 256], F32, tag="psS0", name="psS0")
        psS1 = ppS.tile([128, 256], F32, tag="psS1", name="psS1")

        for i in range(nchunk):
            ch = slice(i * C, (i + 1) * C)
            k0 = planes[("k", 0)][:, ch]
            k1 = planes[("k", 1)][:, ch]
            q0 = planes[("q", 0)][:, ch]
            q1 = planes[("q", 1)][:, ch]

            # token scalars -> [128, {bk, nbk2, aq}]
            psR = ppt.tile([128, 4], F32R, tag="ps", name="psR")
            nc.tensor.transpose(psR[:, 0:4], rows4[0:4, ch], ident[0:4, 0:4])
            tok = ptok.tile([128, 4], F32, tag="tok", name="tok")
            nc.vector.tensor_copy(tok[:, 0:3], psR[:, 0:3])
            if dbg:
                nc.sync.dma_start(out=dbg_d["tok"].ap()[:, i * 4 : (i + 1) * 4], in_=tok)
            if dbg:
                nc.sync.dma_start(out=dbg_d["tok"].ap()[:, i * 4 : (i + 1) * 4], in_=tok)
            bk_t = tok[:, 0:1]
            nbk2_t = tok[:, 1:2]
            aq_t = tok[:, 2:3]

            if i > 0:
                S_sb = pS.tile([128, 512], F32R, tag="S", name="S_sb")
                nc.scalar.activation(out=S_sb[:, 0:256], in_=psS0, func=AF.Copy)
                nc.scalar.activation(out=S_sb[:, 256:512], in_=psS1, func=AF.Copy)

            # A'' = Kc Kc^T ; X = (A''*nbk2) * mlow   (strictly-lower, negated)
            psA = ppt.tile([128, 128], F32, tag="ps", name="psA")
            nc.tensor.matmul(psA, k0, k0, start=True, stop=False)
            nc.tensor.matmul(psA, k1, k1, start=False, stop=True)
            X = pcs.tile([128, 128], F32R, tag="X", name="X")
            nc.vector.scalar_tensor_tensor(
                out=X, in0=psA, scalar=nbk2_t, in1=mlow, op0=OP.mult, op1=OP.mult
            )
            psZ = ppt.tile([128, 128], F32R, tag="ps", name="psZ")
            nc.tensor.transpose(psZ, X, ident)
            Z = pcs.tile([128, 128], F32R, tag="Z", name="Z")
            nc.scalar.activation(out=Z, in_=psZ, func=AF.Copy)

            # power chain: X2, Z2, X4, Z4, Z8
            def sqmm(lhsT, rhs, name, eng):
                psp = ppt.tile([128, 128], F32, tag="ps", name=f"psp_{name}")
                nc.tensor.matmul(psp, lhsT, rhs, start=True, stop=True)
                t_ = pcs.tile([128, 128], F32R, tag=name, name=name)
                if eng == "s":
                    nc.scalar.activation(out=t_, in_=psp, func=AF.Copy)
                else:
                    nc.vector.tensor_copy(t_, psp)
                return t_

            X2 = sqmm(Z, X, "X2", "s")
            Z2 = sqmm(X, Z, "Z2", "v")
            X4 = sqmm(Z2, X2, "X4", "s")
            Z4 = sqmm(X2, Z2, "Z4", "v")
            Z8 = sqmm(X4, Z4, "Z8", "s")

            # V transpose + vb = bk * v_tok
            psV = ppt.tile([128, 256], F32R, tag="ps", name="psV")
            nc.tensor.transpose(psV[:, 0:128], planes[("v", 0)][:, ch], ident)
            nc.tensor.transpose(psV[:, 128:256], planes[("v", 1)][:, ch], ident)
            vb = pcm.tile([128, 256], F32R, tag="vb", name="vb")
            nc.vector.tensor_scalar(
                out=vb, in0=psV, scalar1=bk_t, scalar2=None, op0=OP.mult
            )

            # y = vb + nbk2 * (K S)
            if i > 0:
                psKS = ppt.tile([128, 256], F32, tag="ps", name="psKS")
                nc.tensor.matmul(psKS, k0, S_sb[:, 0:256], start=True, stop=False)
                nc.tensor.matmul(psKS, k1, S_sb[:, 256:512], start=False, stop=True)
                y = pcm.tile([128, 256], F32R, tag="y", name="y")
                nc.vector.scalar_tensor_tensor(
                    out=y, in0=psKS, scalar=nbk2_t, in1=vb, op0=OP.mult, op1=OP.add
                )
            else:
                y = vb

            # t-chain: w = (I+X)(I+X2)(I+X4)(I+X8) y ; lhsT = Z8,Z4,Z2,Z
            cur = y
            for idx, P in enumerate((Z8, Z4, Z2, Z)):
                pst = ppt.tile([128, 256], F32, tag="ps", name=f"pst{idx}")
                nc.tensor.matmul(pst, P, cur, start=True, stop=False)
                nc.tensor.matmul(pst, ident, cur, start=False, stop=True)
                nxt = pcm.tile([128, 256], F32R, tag=f"t{idx}", name=f"t{idx}")
                if idx % 2 == 0:
                    nc.scalar.activation(out=nxt, in_=pst, func=AF.Copy)
                else:
                    nc.vector.tensor_copy(nxt, pst)
                cur = nxt
            w = cur

            if dbg:
                nc.sync.dma_start(out=dbg_d["w"].ap()[:, i*256:(i+1)*256], in_=w[:, :].bitcast(F32))
                nc.sync.dma_start(out=dbg_d["X"].ap()[:, i*128:(i+1)*128], in_=X[:, :].bitcast(F32))
                if i > 0:
                    nc.sync.dma_start(out=dbg_d["S"].ap()[:, i*512:(i+1)*512], in_=S_sb[:, :].bitcast(F32))

            # H~m = (K Q^T) * mup  (== G^T with G = tril(QK^T))
            psH = ppt.tile([128, 128], F32, tag="ps", name="psH")
            nc.tensor.matmul(psH, k0, q0, start=True, stop=False)
            nc.tensor.matmul(psH, k1, q1, start=False, stop=True)
            Hm = pcs.tile([128, 128], F32R, tag="Hm", name="Hm")
            nc.vector.tensor_mul(Hm, psH, mup)

            # o = Q S + Hm^T w
            pso = ppt.tile([128, 256], F32, tag="ps", name="pso")
            if i > 0:
                nc.tensor.matmul(pso, q0, S_sb[:, 0:256], start=True, stop=False)
                nc.tensor.matmul(pso, q1, S_sb[:, 256:512], start=False, stop=False)
                nc.tensor.matmul(pso, Hm, w, start=False, stop=True)
            else:
                nc.tensor.matmul(pso, Hm, w, start=True, stop=True)

            # rms-norm + alpha_q scale
            sums = ptok.tile([128, 1], F32, tag="sums", name="sums")
            scratch = pcm.tile([128, 256], F32, tag="scr", name="scratch")
            nc.scalar.activation(out=scratch, in_=pso, func=AF.Square, accum_out=sums)
            aq2 = ptok.tile([128, 1], F32, tag="aq2", name="aq2")
            nc.vector.tensor_scalar(
                out=aq2,
                in0=aq_t,
                scalar1=aq_t,
                scalar2=1.0 / D,
                op0=OP.mult,
                op1=OP.mult,
            )
            rstd_t = ptok.tile([128, 1], F32, tag="rstd_t", name="rstd_t")
            nc.scalar.activation(
                out=rstd_t, in_=sums, func=AF.Sqrt, scale=aq2[:, 0:1], bias=eps5
            )
            rstd = ptok.tile([128, 1], F32, tag="rstd", name="rstd")
            nc.vector.reciprocal(out=rstd, in_=rstd_t)
            on = pcm.tile([128, 256], F32R, tag="on", name="on")
            nc.vector.tensor_scalar(
                out=on,
                in0=pso,
                scalar1=rstd[:, 0:1],
                scalar2=aq_t,
                op0=OP.mult,
                op1=OP.mult,
            )

            if dbg:
                nc.sync.dma_start(out=dbg_d["on"].ap()[:, i*256:(i+1)*256], in_=on[:, :].bitcast(F32))

            # ktok (K chunk transposed) + state update  S += K^T w
            psK = ppt.tile([128, 256], F32R, tag="ps", name="psK")
            nc.tensor.transpose(psK[:, 0:128], k0, ident)
            nc.tensor.transpose(psK[:, 128:256], k1, ident)
            ktok = pcm.tile([128, 256], F32R, tag="ktok", name="ktok")
            nc.scalar.activation(out=ktok, in_=psK, func=AF.Copy)
            nc.tensor.matmul(
                psS0,
                ktok[:, 0:128],
                w,
                start=(i == 0),
                stop=(i == nchunk - 1),
                skip_group_check=True,
            )
            nc.tensor.matmul(
                psS1,
                ktok[:, 128:256],
                w,
                start=(i == 0),
                stop=(i == nchunk - 1),
                skip_group_check=True,
            )

            # o transpose + partial o_proj
            psOT = ppt.tile([128, 256], F32R, tag="ps", name="psOT")
            nc.tensor.transpose(psOT[:, 0:128], on[:, 0:128], ident)
            nc.tensor.transpose(psOT[:, 128:256], on[:, 128:256], ident)
            ot = pcm.tile([128, 256], F32R, tag="ot", name="ot")
            nc.scalar.activation(out=ot, in_=psOT, func=AF.Copy)
            outbuf = pout.tile([128, HID], F32, tag="outbuf", name="outbuf")
            for hc in range(2):
                psop = ppop.tile([128, 512], F32, tag="op", name="psop")
                nc.tensor.matmul(
                    psop,
                    ot[:, 0:128],
                    wo_sb[:, 0, hc * 512 : (hc + 1) * 512],
                    start=True,
                    stop=False,
                )
                nc.tensor.matmul(
                    psop,
                    ot[:, 128:256],
                    wo_sb[:, 1, hc * 512 : (hc + 1) * 512],
                    start=False,
                    stop=True,
                )
                if hc == 0:
                    nc.vector.tensor_copy(outbuf[:, 0:512], psop)
                else:
                    nc.scalar.activation(
                        out=outbuf[:, 512:1024], in_=psop, func=AF.Copy
                    )
            nc.sync.dma_start(out=out_d.ap()[ch, :], in_=outbuf)

    nc.compile()
    return nc


def make_host_inputs(inputs, nchunk=S_FULL // C):
    """Shard + preprocess full inputs into per-core in_maps."""
    S = nchunk * C
    hs = np.ascontiguousarray(np.asarray(inputs["hidden_states"])[:, :S, :]).astype(
        np.float32
    )
    Wq, Wk, Wv = (np.asarray(inputs[k], np.float32) for k in ("Wq", "Wk", "Wv"))
    Wb = np.asarray(inputs["Wb"], np.float32)
    Wo = np.asarray(inputs["Wo"], np.float32)
    nw = np.asarray(inputs["norm_w"], np.float32)
    convs = {
        k: np.asarray(inputs[k], np.float32) for k in ("conv_q", "conv_k", "conv_v")
    }

    ident = np.eye(128, dtype=np.float32)
    onescol = np.ones((128, 1), np.float32)
    mlow = np.tril(np.ones((128, 128), np.float32), -1)
    mup = np.triu(np.ones((128, 128), np.float32), 0)

    def diag_pack(cw):
        # cw: [256, 4] tap weights for this head -> [128, 8*128]
        out = np.zeros((128, 8 * 128), np.float32)
        for j in range(4):
            for dt_ in range(2):
                blk = np.diag(cw[dt_ * 128 : (dt_ + 1) * 128, j])
                out[:, (j * 2 + dt_) * 128 : (j * 2 + dt_ + 1) * 128] = blk
        return out

    in_maps = []
    for core in range(8):
        b, h = core // 4, core % 4
        hsel = slice(h * D, (h + 1) * D)
        in_maps.append(
            {
                "xt": np.ascontiguousarray(hs[b].T),
                "wq": np.ascontiguousarray(Wq[:, hsel]),
                "wk": np.ascontiguousarray(Wk[:, hsel]),
                "wv": np.ascontiguousarray(Wv[:, hsel]),
                "wb": np.ascontiguousarray(Wb[:, h : h + 1]),
                "wo": np.ascontiguousarray(nw[:, None] * Wo[hsel, :]),
                "cdq": diag_pack(convs["conv_q"][hsel]),
                "cdk": diag_pack(convs["conv_k"][hsel]),
                "cdv": diag_pack(convs["conv_v"][hsel]),
                "ident": ident,
                "onescol": onescol,
                "mlow": mlow,
                "mup": mup,
            }
        )
    return in_maps


_NC_CACHE = {}


def _get_nc(nchunk):
    if nchunk not in _NC_CACHE:
        _NC_CACHE[nchunk] = build_nc(nchunk)
    return _NC_CACHE[nchunk]


def kernel(**inputs) -> np.ndarray:
    nchunk = S_FULL // C
    nc = _get_nc(nchunk)
    in_maps = make_host_inputs(inputs, nchunk)
    res = run_bass_kernel_spmd(nc, in_maps, core_ids=list(range(8)))
    S = nchunk * C
    out = np.zeros((B, S, HID), np.float32)
    for core in range(8):
        out[core // 4] += res.results[core]["out"]
    return out
